# revision 14
# baseline (speedup 1.0000x reference)
"""ChebConv (K=5) Trainium2 Bass kernel.

Math (reference): for the Laplacian L given by COO (rows, cols, vals) over V
nodes, with node features x0 [V, D] (D = Fin*B*X*Y*Z, reordered per core):
    T_0 = x0 ; T_1 = L x0 ; T_k = 2 L T_{k-1} - T_{k-2}
    out[vs, fout] = sum_k sum_fin T_k[vs, fin] W[k, fin, fout] + bias

Sharding: data-parallel over the (B, X) "combo" grid -> 16 combos, 2 per core.
Per-core feature width D_local = 1024, column order (s_hi=(combo,y) 8,
s_lo=z 4, fin 32).

Device algorithm per core:
  - SpMM level k (k = 1..4): edges sorted by 128-row destination block, with
    per-block source dedup into "slots" (padded to multiples of 128).
    dma_gather pulls slot rows of T_{k-1} (bf16 [4096, 1024], HBM) into SBUF
    G tiles [128 slots, nch, 1024]; PE matmuls accumulate
    y_R = sum_ch S_ch^T @ G_ch into PSUM (S holds 2*vals; level 1 rescaled by
    0.5 at evacuation), plus one (-I) @ T_{k-2}[R] matmul for the recurrence.
    ACT evacuates PSUM -> SBUF bf16, DMA stores T_k to HBM.
  - GEMM: per v-chunk, DMA-transpose loads TT_k [128=(s_lo,fin), (v,s_hi)]
    from T_k; 5 accumulating K=32 matmuls per (s_lo, n-chunk) with W_k;
    DVE adds bias; DMA stores out^T [s_lo, fout, (v, s_hi)] fp32.

Host reassembles the full [B, Fout, V, X, Y, Z] output.
"""

import numpy as np
import ml_dtypes

import concourse.bass as bass
import concourse.mybir as mybir
import concourse.tile as tile
from concourse import bacc
from concourse.bass_utils import run_bass_kernel_spmd

B, Fin, Fout, V, X, Y, Z, K, NNZ = 4, 32, 64, 4096, 4, 4, 4, 5, 28672
NCORES = 8
COMBOS_PER_CORE = (B * X) // NCORES  # 2
SH = COMBOS_PER_CORE * Y  # 8  (combo, y)
SL = Z  # 4
D = SH * SL * Fin  # 1024
BLK = 128
NBLK = V // BLK  # 32
NS = 512  # matmul free-dim slice
GATHER_BLKS = 1  # dest blocks per dma_gather call (num_idxs>1024 crashed HW)
VCHUNK = 1024  # GEMM v-chunk
bf16 = ml_dtypes.bfloat16

_last_results = None  # BassKernelResults of the most recent run (for test.py)


def _build_graph_data(rows, cols, vals):
    """Slot lists + S chunks, dest-block sorted, per-block source dedup.

    Returns (slot_idx int32 [NSLOT], s_chunks fp32 [NCH, 128, 128],
             nch_per_block list[int]).  S entries hold 2*vals.
    """
    rows = np.asarray(rows).astype(np.int64)
    cols = np.asarray(cols).astype(np.int64)
    vals = np.asarray(vals).astype(np.float64)
    order = np.argsort(rows, kind="stable")
    r_s, c_s, v_s = rows[order], cols[order], vals[order]
    slot_idx, s_chunks, nch_per_block = [], [], []
    for Rb in range(NBLK):
        m = (r_s // BLK) == Rb
        rb, cb, vb = r_s[m] - Rb * BLK, c_s[m], v_s[m]
        uniq, inv = np.unique(cb, return_inverse=True)
        nu = len(uniq)
        nch = max(1, (nu + BLK - 1) // BLK)
        nslots = nch * BLK
        slots = np.zeros(nslots, np.int64)
        slots[:nu] = uniq  # pad slots gather row 0; S columns stay zero
        S = np.zeros((nslots, BLK), np.float64)
        np.add.at(S, (inv, rb), 2.0 * vb)
        slot_idx.append(slots)
        s_chunks.append(S.reshape(nch, BLK, BLK))
        nch_per_block.append(nch)
    return (
        np.concatenate(slot_idx).astype(np.int32),
        np.concatenate(s_chunks, 0).astype(np.float32),
        nch_per_block,
    )


def _wrap_idx16(slot_idx):
    """int16 index tensor [128, n//16]: index j at partition j%16 + 16k
    (replicated over the 8 gpsimd cores), free position j//16."""
    n = len(slot_idx)
    assert n % 16 == 0
    w = slot_idx.astype(np.int16).reshape(n // 16, 16).T  # [16, n//16]
    return np.tile(w, (8, 1))  # [128, n//16]


def _pad_pairs(slot_idx, nch_per_block):
    """Pad each GATHER_BLKS-block group's slot list to the max group size so
    every dma_gather call has identical num_idxs (one shared register)."""
    nch_base = np.concatenate([[0], np.cumsum(nch_per_block)]).astype(int)
    pair_nch = [
        sum(nch_per_block[j:j + GATHER_BLKS])
        for j in range(0, NBLK, GATHER_BLKS)
    ]
    max_nch = max(pair_nch)
    nidx_call = max_nch * 128
    out = np.zeros((len(pair_nch), nidx_call), np.int32)
    for p, j in enumerate(range(0, NBLK, GATHER_BLKS)):
        s0, s1 = int(nch_base[j]) * 128, int(nch_base[min(j + GATHER_BLKS, NBLK)]) * 128
        out[p, : s1 - s0] = slot_idx[s0:s1]
    return out.reshape(-1), nidx_call


def _build_program(nch_per_block, nslot, nidx_call):
    nch_base = np.concatenate([[0], np.cumsum(nch_per_block)]).astype(int)
    NCH = int(nch_base[-1])
    fp32, bft, i16 = mybir.dt.float32, mybir.dt.bfloat16, mybir.dt.int16

    nc = bacc.Bacc("TRN2", target_bir_lowering=False, debug=False,
                   enable_asserts=False, num_devices=NCORES)
    x0 = nc.dram_tensor("x0", [V, D], bft, kind="ExternalInput").ap()
    sv = nc.dram_tensor("sv", [128, NCH * 128], bft, kind="ExternalInput").ap()
    gi = nc.dram_tensor("gi", [128, nslot // 16], i16, kind="ExternalInput").ap()
    wk = nc.dram_tensor("wk", [128, K * Fout], bft, kind="ExternalInput").ap()
    ni = nc.dram_tensor("ni", [128, 128], bft, kind="ExternalInput").ap()
    bs = nc.dram_tensor("bs", [Fout, 1], fp32, kind="ExternalInput").ap()
    out = nc.dram_tensor("out", [SL, Fout, V * SH], fp32,
                         kind="ExternalOutput").ap()

    Copy = mybir.ActivationFunctionType.Copy
    max_pair_nch = nidx_call // 128

    with tile.TileContext(nc) as tc:
        with tc.tile_pool(name="dram", bufs=1, space="DRAM") as dpool:
            tdram = [x0]
            for k in range(1, K):
                tdram.append(
                    dpool.tile([V, D], bft, tag=f"tk{k}", name=f"tk{k}"))

            with tc.tile_pool(name="const", bufs=1) as cpool:
                sv_sb = cpool.tile([128, NCH * 128], bft, name="sv_sb")
                nc.sync.dma_start(out=sv_sb, in_=sv)
                gi_sb = cpool.tile([128, nslot // 16], i16, name="gi_sb")
                nc.sync.dma_start(out=gi_sb, in_=gi)
                ni_sb = cpool.tile([128, 128], bft, name="ni_sb")
                nc.sync.dma_start(out=ni_sb, in_=ni)
                wk_sb = cpool.tile([128, K * Fout], bft, name="wk_sb")
                nc.sync.dma_start(out=wk_sb, in_=wk)
                bs_sb = cpool.tile([Fout, 1], fp32, name="bs_sb")
                nc.sync.dma_start(out=bs_sb, in_=bs)

                # ---- SpMM recurrence levels ----
                nidx_reg = nc.gpsimd.to_reg(nidx_call)
                with tc.tile_pool(name="lvl", bufs=2) as lpool, \
                     tc.tile_pool(name="lps", bufs=4, space="PSUM") as ppool:
                    for k in range(1, K):
                        src = tdram[k - 1]
                        for pj, j0 in enumerate(range(0, NBLK, GATHER_BLKS)):
                            blks = range(j0, min(j0 + GATHER_BLKS, NBLK))
                            s0 = pj * nidx_call
                            g = lpool.tile([128, max_pair_nch, D], bft,
                                           tag="g", name=f"g_{k}_{j0}")
                            nc.gpsimd.dma_gather(
                                g, src,
                                gi_sb[:, s0 // 16:(s0 + nidx_call) // 16],
                                nidx_call, nidx_reg, D, elem_step=D,
                            )
                            for R in blks:
                                nch = nch_per_block[R]
                                po = int(nch_base[R] - nch_base[j0])
                                ps = [
                                    ppool.tile([128, NS], fp32, tag="ps",
                                               name=f"ps_{k}_{R}_{s}")
                                    for s in range(D // NS)
                                ]
                                for s in range(D // NS):
                                    for c in range(nch):
                                        cb = int(nch_base[R]) + c
                                        nc.tensor.matmul(
                                            ps[s],
                                            lhsT=sv_sb[:, cb * 128:(cb + 1) * 128],
                                            rhs=g[:, po + c, s * NS:(s + 1) * NS],
                                            start=(c == 0),
                                            stop=(k == 1 and c == nch - 1),
                                        )
                                if k >= 2:
                                    tm2 = lpool.tile([128, D], bft, tag="tm2",
                                                     name=f"tm2_{k}_{R}")
                                    nc.sync.dma_start(
                                        out=tm2,
                                        in_=tdram[k - 2][R * BLK:(R + 1) * BLK, :])
                                    for s in range(D // NS):
                                        nc.tensor.matmul(
                                            ps[s], lhsT=ni_sb,
                                            rhs=tm2[:, s * NS:(s + 1) * NS],
                                            start=False, stop=True,
                                        )
                                tkb = lpool.tile([128, D], bft, tag="tkb",
                                                 name=f"tkb_{k}_{R}")
                                for s in range(D // NS):
                                    nc.scalar.activation(
                                        tkb[:, s * NS:(s + 1) * NS], ps[s],
                                        Copy, scale=(0.5 if k == 1 else 1.0))
                                nc.sync.dma_start(
                                    out=tdram[k][R * BLK:(R + 1) * BLK, :],
                                    in_=tkb)

                # ---- GEMM ----
                with tc.tile_pool(name="gem", bufs=1) as gpool, \
                     tc.tile_pool(name="gob", bufs=2) as opool, \
                     tc.tile_pool(name="gps", bufs=4, space="PSUM") as qpool:
                    for vc in range(V // VCHUNK):
                        tts = []
                        for k in range(K):
                            tt = gpool.tile([128, VCHUNK * SH], bft,
                                            tag=f"tt{k}", name=f"tt{k}_{vc}")
                            src2d = tdram[k].rearrange(
                                "v (h c) -> (v h) c", h=SH)
                            nc.sync.dma_start(
                                out=tt,
                                in_=src2d[vc * VCHUNK * SH:(vc + 1) * VCHUNK * SH, :],
                                transpose=True)
                            tts.append(tt)
                        npos = VCHUNK * SH  # 8192
                        for sl in range(SL):
                            ob = opool.tile([Fout, npos], fp32, tag="ob",
                                            name=f"ob_{vc}_{sl}")
                            for n0 in range(0, npos, NS):
                                q = qpool.tile([Fout, NS], fp32, tag="q",
                                               name=f"q_{vc}_{sl}_{n0}")
                                for k in range(K):
                                    nc.tensor.matmul(
                                        q,
                                        lhsT=wk_sb[32 * sl:32 * sl + 32,
                                                   k * Fout:(k + 1) * Fout],
                                        rhs=tts[k][32 * sl:32 * sl + 32,
                                                   n0:n0 + NS],
                                        start=(k == 0), stop=(k == K - 1),
                                        tile_position=(32 * sl, 0),
                                    )
                                nc.vector.tensor_tensor(
                                    out=ob[:, n0:n0 + NS], in0=q,
                                    in1=bs_sb.to_broadcast([Fout, NS]),
                                    op=mybir.AluOpType.add)
                            nc.sync.dma_start(
                                out=out[sl, :, vc * npos:(vc + 1) * npos],
                                in_=ob)
    nc.compile()
    return nc


def _host_prep(x, vals, weight, bias, rows, cols):
    """Returns (nc, in_maps) for the 8 cores."""
    slot_idx, s_chunks, nch_per_block = _build_graph_data(rows, cols, vals)
    padded_idx, nidx_call = _pad_pairs(slot_idx, nch_per_block)
    nslot = len(padded_idx)
    nc = _build_program(nch_per_block, nslot, nidx_call)

    # svals as lhsT chunks: [128 K-slots, chunk-major * 128 dest cols]
    NCH = s_chunks.shape[0]
    sv_np = np.ascontiguousarray(
        s_chunks.transpose(1, 0, 2).reshape(128, NCH * 128)).astype(bf16)
    gi_np = _wrap_idx16(padded_idx)
    # W [K, Fin, Fout] -> [128=(4 s-copies x 32 fin), K*Fout] replicated
    wk_np = np.tile(
        np.asarray(weight).transpose(1, 0, 2).reshape(Fin, K * Fout),
        (4, 1)).astype(bf16)
    ni_np = (-np.eye(128, dtype=np.float32)).astype(bf16)
    bs_np = np.asarray(bias, np.float32).reshape(Fout, 1)

    # x [B, Fin, V, X, Y, Z] -> [V, combo=(B,X), Y, Z, Fin]
    xt = np.transpose(np.asarray(x), (2, 0, 3, 4, 5, 1)).reshape(
        V, B * X, Y, Z, Fin)
    in_maps = []
    for c in range(NCORES):
        x0_np = np.ascontiguousarray(
            xt[:, COMBOS_PER_CORE * c:COMBOS_PER_CORE * (c + 1)]
        ).reshape(V, D).astype(bf16)
        in_maps.append({
            "x0": x0_np, "sv": sv_np, "gi": gi_np,
            "wk": wk_np, "ni": ni_np, "bs": bs_np,
        })
    return nc, in_maps


def kernel(x, vals, weight, bias, rows, cols):
    global _last_results
    nc, in_maps = _host_prep(x, vals, weight, bias, rows, cols)
    res = run_bass_kernel_spmd(nc, in_maps, core_ids=list(range(NCORES)))
    _last_results = res
    # Reassemble: per-core out [SL=z, Fout, (v, s_hi=(combo, y))] fp32
    out_full = np.empty((B, Fout, V, X, Y, Z), np.float32)
    for c in range(NCORES):
        oc = res.results[c]["out"].reshape(SL, Fout, V, COMBOS_PER_CORE, Y)
        # -> [combo, Fout, V, Y, Z]
        oc = np.transpose(oc, (3, 1, 2, 4, 0))
        for ci in range(COMBOS_PER_CORE):
            combo = COMBOS_PER_CORE * c + ci
            b, xx = divmod(combo, X)
            out_full[b, :, :, xx] = oc[ci]
    return out_full


# revision 18
# speedup vs baseline: 1041.5772x; 1041.5772x over previous
"""ChebConv (K=5) Trainium2 Bass kernel.

Math (reference): for the Laplacian L given by COO (rows, cols, vals) over V
nodes, with node features x0 [V, D] (D = Fin*B*X*Y*Z, reordered per core):
    T_0 = x0 ; T_1 = L x0 ; T_k = 2 L T_{k-1} - T_{k-2}
    out[vs, fout] = sum_k sum_fin T_k[vs, fin] W[k, fin, fout] + bias

Sharding: data-parallel over the (B, X) "combo" grid -> 16 combos, 2 per core.
Per-core feature width D_local = 1024, column order (s_hi=(combo,y) 8,
s_lo=z 4, fin 32).

Device algorithm per core:
  - SpMM level k (k = 1..4): edges sorted by 128-row destination block, with
    per-block source dedup into "slots" (padded to multiples of 128).
    dma_gather pulls slot rows of T_{k-1} (bf16 [4096, 1024], HBM) into SBUF
    G tiles [128 slots, nch, 1024]; PE matmuls accumulate
    y_R = sum_ch S_ch^T @ G_ch into PSUM (S holds 2*vals; level 1 rescaled by
    0.5 at evacuation), plus one (-I) @ T_{k-2}[R] matmul for the recurrence.
    ACT evacuates PSUM -> SBUF bf16, DMA stores T_k to HBM.
  - GEMM: per v-chunk, DMA-transpose loads TT_k [128=(s_lo,fin), (v,s_hi)]
    from T_k; 5 accumulating K=32 matmuls per (s_lo, n-chunk) with W_k;
    DVE adds bias; DMA stores out^T [s_lo, fout, (v, s_hi)] fp32.

Host reassembles the full [B, Fout, V, X, Y, Z] output.
"""

import numpy as np
import ml_dtypes

import concourse.bass as bass
import concourse.mybir as mybir
import concourse.tile as tile
from concourse import bacc
from concourse.bass_utils import run_bass_kernel_spmd

B, Fin, Fout, V, X, Y, Z, K, NNZ = 4, 32, 64, 4096, 4, 4, 4, 5, 28672
NCORES = 8
COMBOS_PER_CORE = (B * X) // NCORES  # 2
SH = COMBOS_PER_CORE * Y  # 8  (combo, y)
SL = Z  # 4
D = SH * SL * Fin  # 1024
BLK = 128
NBLK = V // BLK  # 32
NS = 512  # matmul free-dim slice
GATHER_BLKS = 1  # dest blocks per dma_gather call (num_idxs>1024 crashed HW)
VCHUNK = 1024  # GEMM v-chunk
bf16 = ml_dtypes.bfloat16

_last_results = None  # BassKernelResults of the most recent run (for test.py)


def _build_graph_data(rows, cols, vals):
    """Slot lists + S chunks, dest-block sorted, per-block source dedup.

    Returns (slot_idx int32 [NSLOT], s_chunks fp32 [NCH, 128, 128],
             nch_per_block list[int]).  S entries hold 2*vals.
    """
    rows = np.asarray(rows).astype(np.int64)
    cols = np.asarray(cols).astype(np.int64)
    vals = np.asarray(vals).astype(np.float64)
    order = np.argsort(rows, kind="stable")
    r_s, c_s, v_s = rows[order], cols[order], vals[order]
    slot_idx, s_chunks, nch_per_block = [], [], []
    for Rb in range(NBLK):
        m = (r_s // BLK) == Rb
        rb, cb, vb = r_s[m] - Rb * BLK, c_s[m], v_s[m]
        uniq, inv = np.unique(cb, return_inverse=True)
        nu = len(uniq)
        nch = max(1, (nu + BLK - 1) // BLK)
        nslots = nch * BLK
        slots = np.zeros(nslots, np.int64)
        slots[:nu] = uniq  # pad slots gather row 0; S columns stay zero
        S = np.zeros((nslots, BLK), np.float64)
        np.add.at(S, (inv, rb), 2.0 * vb)
        slot_idx.append(slots)
        s_chunks.append(S.reshape(nch, BLK, BLK))
        nch_per_block.append(nch)
    return (
        np.concatenate(slot_idx).astype(np.int32),
        np.concatenate(s_chunks, 0).astype(np.float32),
        nch_per_block,
    )


def _wrap_idx16(slot_idx):
    """int16 index tensor [128, n//16]: index j at partition j%16 + 16k
    (replicated over the 8 gpsimd cores), free position j//16."""
    n = len(slot_idx)
    assert n % 16 == 0
    w = slot_idx.astype(np.int16).reshape(n // 16, 16).T  # [16, n//16]
    return np.tile(w, (8, 1))  # [128, n//16]


def _pad_pairs(slot_idx, nch_per_block):
    """Pad each GATHER_BLKS-block group's slot list to the max group size so
    every dma_gather call has identical num_idxs (one shared register)."""
    nch_base = np.concatenate([[0], np.cumsum(nch_per_block)]).astype(int)
    pair_nch = [
        sum(nch_per_block[j:j + GATHER_BLKS])
        for j in range(0, NBLK, GATHER_BLKS)
    ]
    max_nch = max(pair_nch)
    nidx_call = max_nch * 128
    out = np.zeros((len(pair_nch), nidx_call), np.int32)
    for p, j in enumerate(range(0, NBLK, GATHER_BLKS)):
        s0, s1 = int(nch_base[j]) * 128, int(nch_base[min(j + GATHER_BLKS, NBLK)]) * 128
        out[p, : s1 - s0] = slot_idx[s0:s1]
    return out.reshape(-1), nidx_call


def _build_program(nch_per_block, nslot, nidx_call, reps=1):
    """reps>1 wraps the whole compute body in a device-side loop — used only
    for benchmarking (amortizes host dispatch overhead)."""
    import contextlib
    nch_base = np.concatenate([[0], np.cumsum(nch_per_block)]).astype(int)
    NCH = int(nch_base[-1])
    fp32, bft, i16 = mybir.dt.float32, mybir.dt.bfloat16, mybir.dt.int16

    nc = bacc.Bacc("TRN2", target_bir_lowering=False, debug=False,
                   enable_asserts=False, num_devices=NCORES)
    x0 = nc.dram_tensor("x0", [V, D], bft, kind="ExternalInput").ap()
    sv = nc.dram_tensor("sv", [128, NCH * 128], bft, kind="ExternalInput").ap()
    gi = nc.dram_tensor("gi", [128, nslot // 16], i16, kind="ExternalInput").ap()
    wk = nc.dram_tensor("wk", [128, K * Fout], bft, kind="ExternalInput").ap()
    ni = nc.dram_tensor("ni", [128, 128], bft, kind="ExternalInput").ap()
    bs = nc.dram_tensor("bs", [Fout, 1], fp32, kind="ExternalInput").ap()
    out = nc.dram_tensor("out", [SL, Fout, V * SH], fp32,
                         kind="ExternalOutput").ap()

    Copy = mybir.ActivationFunctionType.Copy
    max_pair_nch = nidx_call // 128

    with tile.TileContext(nc) as tc:
        with tc.tile_pool(name="dram", bufs=1, space="DRAM") as dpool:
            tdram = [x0]
            for k in range(1, K):
                tdram.append(
                    dpool.tile([V, D], bft, tag=f"tk{k}", name=f"tk{k}"))

            with tc.tile_pool(name="const", bufs=1) as cpool:
                sv_sb = cpool.tile([128, NCH * 128], bft, name="sv_sb")
                nc.sync.dma_start(out=sv_sb, in_=sv)
                gi_sb = cpool.tile([128, nslot // 16], i16, name="gi_sb")
                nc.sync.dma_start(out=gi_sb, in_=gi)
                ni_sb = cpool.tile([128, 128], bft, name="ni_sb")
                nc.sync.dma_start(out=ni_sb, in_=ni)
                wk_sb = cpool.tile([128, K * Fout], bft, name="wk_sb")
                nc.sync.dma_start(out=wk_sb, in_=wk)
                bs_sb = cpool.tile([Fout, 1], fp32, name="bs_sb")
                nc.sync.dma_start(out=bs_sb, in_=bs)

                # ---- SpMM recurrence levels ----
                nidx_reg = nc.gpsimd.to_reg(nidx_call)
                rep_cm = tc.For_i(0, reps, 1) if reps > 1 else contextlib.nullcontext()
                with rep_cm:
                    _run_body(nc, tc, tdram, out, sv_sb, gi_sb, ni_sb, wk_sb,
                              bs_sb, nidx_reg, nch_per_block, nch_base,
                              nidx_call)
    nc.compile()
    return nc


def _run_body(nc, tc, tdram, out, sv_sb, gi_sb, ni_sb, wk_sb, bs_sb,
              nidx_reg, nch_per_block, nch_base, nidx_call):
    fp32, bft = mybir.dt.float32, mybir.dt.bfloat16
    Copy = mybir.ActivationFunctionType.Copy
    max_pair_nch = nidx_call // 128
    if True:
        if True:
                with tc.tile_pool(name="lvl", bufs=2) as lpool, \
                     tc.tile_pool(name="lps", bufs=4, space="PSUM") as ppool:
                    for k in range(1, K):
                        src = tdram[k - 1]
                        for pj, j0 in enumerate(range(0, NBLK, GATHER_BLKS)):
                            blks = range(j0, min(j0 + GATHER_BLKS, NBLK))
                            s0 = pj * nidx_call
                            g = lpool.tile([128, max_pair_nch, D], bft,
                                           tag="g", name=f"g_{k}_{j0}")
                            nc.gpsimd.dma_gather(
                                g, src,
                                gi_sb[:, s0 // 16:(s0 + nidx_call) // 16],
                                nidx_call, nidx_reg, D, elem_step=D,
                            )
                            for R in blks:
                                nch = nch_per_block[R]
                                po = int(nch_base[R] - nch_base[j0])
                                ps = [
                                    ppool.tile([128, NS], fp32, tag="ps",
                                               name=f"ps_{k}_{R}_{s}")
                                    for s in range(D // NS)
                                ]
                                for s in range(D // NS):
                                    for c in range(nch):
                                        cb = int(nch_base[R]) + c
                                        nc.tensor.matmul(
                                            ps[s],
                                            lhsT=sv_sb[:, cb * 128:(cb + 1) * 128],
                                            rhs=g[:, po + c, s * NS:(s + 1) * NS],
                                            start=(c == 0),
                                            stop=(k == 1 and c == nch - 1),
                                        )
                                if k >= 2:
                                    tm2 = lpool.tile([128, D], bft, tag="tm2",
                                                     name=f"tm2_{k}_{R}")
                                    nc.sync.dma_start(
                                        out=tm2,
                                        in_=tdram[k - 2][R * BLK:(R + 1) * BLK, :])
                                    for s in range(D // NS):
                                        nc.tensor.matmul(
                                            ps[s], lhsT=ni_sb,
                                            rhs=tm2[:, s * NS:(s + 1) * NS],
                                            start=False, stop=True,
                                        )
                                tkb = lpool.tile([128, D], bft, tag="tkb",
                                                 name=f"tkb_{k}_{R}")
                                for s in range(D // NS):
                                    nc.scalar.activation(
                                        tkb[:, s * NS:(s + 1) * NS], ps[s],
                                        Copy, scale=(0.5 if k == 1 else 1.0))
                                nc.sync.dma_start(
                                    out=tdram[k][R * BLK:(R + 1) * BLK, :],
                                    in_=tkb)

                # ---- GEMM ----
                with tc.tile_pool(name="gem", bufs=1) as gpool, \
                     tc.tile_pool(name="gob", bufs=2) as opool, \
                     tc.tile_pool(name="gps", bufs=4, space="PSUM") as qpool:
                    for vc in range(V // VCHUNK):
                        tts = []
                        for k in range(K):
                            tt = gpool.tile([128, VCHUNK * SH], bft,
                                            tag=f"tt{k}", name=f"tt{k}_{vc}")
                            src2d = tdram[k].rearrange(
                                "v (h c) -> (v h) c", h=SH)
                            nc.sync.dma_start(
                                out=tt,
                                in_=src2d[vc * VCHUNK * SH:(vc + 1) * VCHUNK * SH, :],
                                transpose=True)
                            tts.append(tt)
                        npos = VCHUNK * SH  # 8192
                        for sl in range(SL):
                            ob = opool.tile([Fout, npos], fp32, tag="ob",
                                            name=f"ob_{vc}_{sl}")
                            for n0 in range(0, npos, NS):
                                q = qpool.tile([Fout, NS], fp32, tag="q",
                                               name=f"q_{vc}_{sl}_{n0}")
                                for k in range(K):
                                    nc.tensor.matmul(
                                        q,
                                        lhsT=wk_sb[32 * sl:32 * sl + 32,
                                                   k * Fout:(k + 1) * Fout],
                                        rhs=tts[k][32 * sl:32 * sl + 32,
                                                   n0:n0 + NS],
                                        start=(k == 0), stop=(k == K - 1),
                                        tile_position=(32 * sl, 0),
                                    )
                                nc.vector.tensor_tensor(
                                    out=ob[:, n0:n0 + NS], in0=q,
                                    in1=bs_sb.to_broadcast([Fout, NS]),
                                    op=mybir.AluOpType.add)
                            nc.sync.dma_start(
                                out=out[sl, :, vc * npos:(vc + 1) * npos],
                                in_=ob)


def _host_prep(x, vals, weight, bias, rows, cols):
    """Returns (nc, in_maps) for the 8 cores."""
    slot_idx, s_chunks, nch_per_block = _build_graph_data(rows, cols, vals)
    padded_idx, nidx_call = _pad_pairs(slot_idx, nch_per_block)
    nslot = len(padded_idx)
    nc = _build_program(nch_per_block, nslot, nidx_call)

    # svals as lhsT chunks: [128 K-slots, chunk-major * 128 dest cols]
    NCH = s_chunks.shape[0]
    sv_np = np.ascontiguousarray(
        s_chunks.transpose(1, 0, 2).reshape(128, NCH * 128)).astype(bf16)
    gi_np = _wrap_idx16(padded_idx)
    # W [K, Fin, Fout] -> [128=(4 s-copies x 32 fin), K*Fout] replicated
    wk_np = np.tile(
        np.asarray(weight).transpose(1, 0, 2).reshape(Fin, K * Fout),
        (4, 1)).astype(bf16)
    ni_np = (-np.eye(128, dtype=np.float32)).astype(bf16)
    bs_np = np.asarray(bias, np.float32).reshape(Fout, 1)

    # x [B, Fin, V, X, Y, Z] -> [V, combo=(B,X), Y, Z, Fin]
    xt = np.transpose(np.asarray(x), (2, 0, 3, 4, 5, 1)).reshape(
        V, B * X, Y, Z, Fin)
    in_maps = []
    for c in range(NCORES):
        x0_np = np.ascontiguousarray(
            xt[:, COMBOS_PER_CORE * c:COMBOS_PER_CORE * (c + 1)]
        ).reshape(V, D).astype(bf16)
        in_maps.append({
            "x0": x0_np, "sv": sv_np, "gi": gi_np,
            "wk": wk_np, "ni": ni_np, "bs": bs_np,
        })
    return nc, in_maps


def kernel(x, vals, weight, bias, rows, cols):
    global _last_results
    nc, in_maps = _host_prep(x, vals, weight, bias, rows, cols)
    res = run_bass_kernel_spmd(nc, in_maps, core_ids=list(range(NCORES)))
    _last_results = res
    # Reassemble: per-core out [SL=z, Fout, (v, s_hi=(combo, y))] fp32
    out_full = np.empty((B, Fout, V, X, Y, Z), np.float32)
    for c in range(NCORES):
        oc = res.results[c]["out"].reshape(SL, Fout, V, COMBOS_PER_CORE, Y)
        # -> [combo, Fout, V, Y, Z]
        oc = np.transpose(oc, (3, 1, 2, 4, 0))
        for ci in range(COMBOS_PER_CORE):
            combo = COMBOS_PER_CORE * c + ci
            b, xx = divmod(combo, X)
            out_full[b, :, :, xx] = oc[ci]
    return out_full
